# revision 15
# baseline (speedup 1.0000x reference)
"""MoE router kernel for Trainium2 (8 NeuronCores, token-parallel).

Strategy
--------
Tokens (batch*seq = 16384) are sharded 2048/core; the tiny [E,H] gate weight is
replicated.  Per core the gating matmul logits.T = W @ x.T runs on the PE with
the weight planes stationary.  x and W are decomposed on the host into two
fp16 planes each (hi + scaled residual), so the three fp16 matmul chains
reconstruct the fp32 product to ~2^-22 relative accuracy while streaming at
full bf16-class PE rate and keeping the DMA volume identical to fp32.

Chains per h-chunk (contraction tile of 128):
  A = xh · wh_st     -> lga  psum, PE cols 0:64    (wh_st = fp16(2^6 w))
  B = xh · wl_st     -> lgbc psum, PE cols 64:128  (wl_st = fp16(2^17 rw))
  C = xl_st · wh_st  -> lgbc psum, PE cols 64:128  (xl_st = fp16(2^11 rx))
A and B share the moving operand and run in concurrent PE column groups.
scaled logits (2^6 x·w) = A + 2^-11 (B + C); the transposes of the two halves
accumulate in PSUM (a transpose is a matmul), and the 2^-6 unscale is folded
into the ScalarE exp's free affine.

The PE stream is software-pipelined so it never waits on DVE/ACT work:
  iteration g emits   chains(g) | transposes(g-1) | stats-matmuls(g-2)
Per group: exp on ScalarE (accumulating the softmax denominator Z), top-2 via
the DVE Max8/MaxIndex8 instructions, routing weights v/(v1+v2).  Histogram
(ones^T @ one_hot(top1)) and prob-mass (1/Z ^T @ exp) reductions over tokens
accumulate in one PSUM tile; log Z / z-loss run once at the end (single ACT
table set switch).  Scalar losses are finished on the host from the 129
partial sums each core emits.
"""

import numpy as np

import concourse.bacc as bacc
import concourse.mybir as mybir
import concourse.tile as tile
from concourse.bass_utils import run_bass_kernel_spmd

NCORES = 8
B, S, H = 4, 4096, 2048
N = B * S                  # 16384 tokens total
E = 64                     # experts
P = 128                    # partitions
NT = N // NCORES           # 2048 tokens per core
GT = 512                   # tokens per group (matmul free dim)
J = GT // P                # 4 token tiles of 128 per group
HCH = H // P               # 16 contraction chunks

F32 = mybir.dt.float32
F16 = mybir.dt.float16
BF16 = mybir.dt.bfloat16
U32 = mybir.dt.uint32
AF = mybir.ActivationFunctionType
ALU = mybir.AluOpType
AX = mybir.AxisListType


def build_module(nt: int = NT, debug: bool = False, repeat: int = 1):
    """Build + compile the per-core Bass module for nt tokens.

    repeat > 1 wraps the whole body in a hardware For_i loop — used only by
    the benchmark to amplify device time above the RPC noise floor.
    """
    ngroups = nt // GT
    nc = bacc.Bacc("TRN2", target_bir_lowering=False, debug=debug)

    xh_d = nc.dram_tensor("xh", [H, nt], F16, kind="ExternalInput")
    xl_d = nc.dram_tensor("xl", [H, nt], F16, kind="ExternalInput")
    wh_d = nc.dram_tensor("wh", [H, E], F16, kind="ExternalInput")
    wl_d = nc.dram_tensor("wl", [H, E], F16, kind="ExternalInput")
    iota_d = nc.dram_tensor("iota", [P, J * E], F32, kind="ExternalInput")
    ones_d = nc.dram_tensor("ones", [P, 1], F32, kind="ExternalInput")
    ident_d = nc.dram_tensor("ident", [P, E], F32, kind="ExternalInput")
    tokout_d = nc.dram_tensor("tokout", [nt, 4], F32, kind="ExternalOutput")
    stats_d = nc.dram_tensor("stats", [1, 2 * E], F32, kind="ExternalOutput")
    z_d = nc.dram_tensor("zout", [P, nt // P], F32, kind="ExternalOutput")

    with tile.TileContext(nc) as tc:
        with (
            tc.tile_pool(name="const", bufs=1) as cpool,
            tc.tile_pool(name="xp", bufs=3) as xpool,
            tc.tile_pool(name="wk", bufs=2) as wpool,
            tc.tile_pool(name="pp", bufs=2, space="PSUM") as ppool,
            tc.tile_pool(name="sp", bufs=1, space="PSUM") as spool,
        ):
            wh_sb = cpool.tile([P, HCH, E], F16)
            nc.sync.dma_start(wh_sb, wh_d.ap().rearrange("(h p) e -> p h e", p=P))
            wl_sb = cpool.tile([P, HCH, E], F16)
            nc.sync.dma_start(wl_sb, wl_d.ap().rearrange("(h p) e -> p h e", p=P))
            iota_sb = cpool.tile([P, J * E], F32)
            ones_sb = cpool.tile([P, 1], F32)
            onesb_sb = cpool.tile([P, 1], BF16)
            ident_sb = cpool.tile([P, E], F32)

            def emit_const_loads():
                # deferred so the first group's x tiles hit the DMA queue first
                nc.sync.dma_start(iota_sb, iota_d.ap())
                nc.sync.dma_start(ones_sb, ones_d.ap())
                nc.gpsimd.memset(onesb_sb, 1.0)
                nc.sync.dma_start(ident_sb, ident_d.ap())

            # persistent (indexed per group; alive until the stats tail)
            zall_sb = cpool.tile([P, ngroups * J], F32)
            rzall_sb = cpool.tile([P, ngroups * J], F32)
            praw_sb = cpool.tile([P, ngroups * J, E], F32)
            mask_sb = cpool.tile([P, ngroups * J, E], BF16)
            stats_sb = cpool.tile([1, 2 * E], F32)

            # one PSUM tile for the cross-token stats: [counts | probsum]
            stats_ps = spool.tile([1, 2 * E], F32)
            cnt_ps = stats_ps[:, 0:E]
            ps_ps = stats_ps[:, E:2 * E]

            xh_view = xh_d.ap().rearrange("(h p) n -> p h n", p=P)
            xl_view = xl_d.ap().rearrange("(h p) n -> p h n", p=P)
            tok_view = tokout_d.ap().rearrange("(g j p) c -> g p j c", j=J, p=P)

            lg_tiles = {}

            def emit_chains(g):
                xh_sb = xpool.tile([P, HCH, GT], F16, tag="xh")
                xl_sb = xpool.tile([P, HCH, GT], F16, tag="xl")
                hh = HCH // 2
                for half in range(2):
                    hs = slice(half * hh, (half + 1) * hh)
                    ns = slice(g * GT, (g + 1) * GT)
                    nc.sync.dma_start(xh_sb[:, hs, :], xh_view[:, hs, ns])
                    nc.sync.dma_start(xl_sb[:, hs, :], xl_view[:, hs, ns])

                lga_ps = ppool.tile([E, GT], F32, tag="lga")
                lgbc_ps = ppool.tile([P, GT], F32, tag="lgbc")
                for h in range(HCH):
                    nc.tensor.matmul(
                        lga_ps, wh_sb[:, h, :], xh_sb[:, h, :],
                        start=(h == 0), stop=(h == HCH - 1), tile_position=(0, 0),
                    )
                    nc.tensor.matmul(
                        lgbc_ps[E:P, :], wl_sb[:, h, :], xh_sb[:, h, :],
                        start=(h == 0), stop=False, tile_position=(0, 64),
                    )
                    nc.tensor.matmul(
                        lgbc_ps[E:P, :], wh_sb[:, h, :], xl_sb[:, h, :],
                        start=False, stop=(h == HCH - 1), tile_position=(0, 64),
                    )
                lg_tiles[g] = (lga_ps, lgbc_ps)

            def emit_epilogue(g):
                lga_ps, lgbc_ps = lg_tiles.pop(g)
                a_sb = wpool.tile([E, GT], F32, tag="a_sb")
                nc.vector.tensor_copy(a_sb, lga_ps)
                bc_sb = wpool.tile([P, GT], F32, tag="bc_sb")
                nc.vector.tensor_scalar_mul(
                    bc_sb[E:P, :], lgbc_ps[E:P, :], float(2.0 ** -11)
                )

                # transposed halves accumulate in PSUM
                tr_ps = ppool.tile([P, J * E], F32, tag="tr")
                for j in range(J):
                    nc.tensor.matmul(
                        tr_ps[:, j * E:(j + 1) * E],
                        a_sb[:, j * P:(j + 1) * P],
                        ident_sb[0:E, :],
                        start=(j == 0), stop=False, is_transpose=True,
                    )
                for j in range(J):
                    nc.tensor.matmul(
                        tr_ps[:, j * E:(j + 1) * E],
                        bc_sb[E:P, j * P:(j + 1) * P],
                        ident_sb[E:P, :],
                        start=False, stop=(j == J - 1), is_transpose=True,
                    )

                # exp (unscale 2^-6) + per-token softmax denominator Z
                for j in range(J):
                    k = g * J + j
                    nc.scalar.activation(
                        praw_sb[:, k, :], tr_ps[:, j * E:(j + 1) * E], AF.Exp,
                        scale=float(2.0 ** -6),
                        accum_out=zall_sb[:, k:k + 1],
                    )

                # hardware top-8 -> top-2 values + indices
                topv_sb = wpool.tile([P, J, 8], F32, tag="topv")
                topi_sb = wpool.tile([P, J, 8], U32, tag="topi")
                for j in range(J):
                    nc.vector.max(topv_sb[:, j, :], praw_sb[:, g * J + j, :])
                    nc.vector.max_index(
                        topi_sb[:, j, :], topv_sb[:, j, :], praw_sb[:, g * J + j, :]
                    )

                i1f_sb = wpool.tile([P, J], F32, tag="i1f")
                nc.vector.tensor_copy(i1f_sb, topi_sb[:, :, 0])
                i2f_sb = wpool.tile([P, J], F32, tag="i2f")
                nc.vector.tensor_copy(i2f_sb, topi_sb[:, :, 1])

                # one-hot(top1) + 1/Z (consumed by the deferred stats matmuls)
                for j in range(J):
                    nc.vector.tensor_scalar(
                        mask_sb[:, g * J + j, :], iota_sb[:, j * E:(j + 1) * E],
                        i1f_sb[:, j:j + 1], None, ALU.is_equal,
                    )
                nc.vector.reciprocal(
                    rzall_sb[:, g * J:(g + 1) * J], zall_sb[:, g * J:(g + 1) * J]
                )

                # routing weights + packed per-token output [w1 w2 i1 i2]
                tok_sb = wpool.tile([P, J, 4], F32, tag="tok")
                v1 = topv_sb[:, :, 0]
                v2 = topv_sb[:, :, 1]
                s_sb = wpool.tile([P, J], F32, tag="s")
                nc.vector.tensor_add(s_sb, v1, v2)
                rs_sb = wpool.tile([P, J], F32, tag="rs")
                nc.vector.reciprocal(rs_sb, s_sb)
                nc.vector.tensor_mul(tok_sb[:, :, 0], v1, rs_sb)
                nc.vector.tensor_mul(tok_sb[:, :, 1], v2, rs_sb)
                nc.vector.tensor_copy(tok_sb[:, :, 2], i1f_sb)
                nc.vector.tensor_copy(tok_sb[:, :, 3], i2f_sb)
                nc.sync.dma_start(tok_view[g], tok_sb)

            def emit_stats(g):
                for j in range(J):
                    k = g * J + j
                    nc.tensor.matmul(
                        cnt_ps, onesb_sb, mask_sb[:, k, :],
                        start=(k == 0), stop=False,
                    )
                    nc.tensor.matmul(
                        ps_ps, rzall_sb[:, k:k + 1], praw_sb[:, k, :],
                        start=False,
                        stop=(g == ngroups - 1 and j == J - 1),
                    )

            def emit_all():
                # software-pipelined emission: PE never waits on DVE/ACT results
                for g in range(ngroups):
                    emit_chains(g)
                    if g == 0:
                        emit_const_loads()
                    if g >= 1:
                        emit_epilogue(g - 1)
                    if g >= 2:
                        emit_stats(g - 2)
                emit_epilogue(ngroups - 1)
                for g in range(max(ngroups - 2, 0), ngroups):
                    emit_stats(g)

                # z-loss finishes on the host from the exported Z values
                nc.sync.dma_start(z_d.ap(), zall_sb)
                nc.vector.tensor_copy(stats_sb[:, 0:E], cnt_ps)
                nc.vector.tensor_copy(stats_sb[:, E:2 * E], ps_ps)
                nc.sync.dma_start(stats_d.ap(), stats_sb)

            if repeat == 1:
                emit_all()
            else:
                with tc.For_i(0, repeat, 1):
                    emit_all()

    nc.compile()
    return nc


def host_prepare(hidden_states, W_gate):
    """Decompose inputs into fp16 planes and shard tokens across cores."""
    x = np.asarray(hidden_states, dtype=np.float32).reshape(-1, H)
    w = np.asarray(W_gate, dtype=np.float32)

    xh = x.astype(np.float16)
    rx = x - xh.astype(np.float32)                       # exact in fp32
    xl = (rx * np.float32(2048.0)).astype(np.float16)    # 2^11 * residual

    wh = (w * np.float32(64.0)).astype(np.float16)       # fp16(2^6 * w)
    rw = w - wh.astype(np.float32) / np.float32(64.0)    # exact in fp32
    wl = (rw * np.float32(2.0 ** 17)).astype(np.float16)

    xhT = np.ascontiguousarray(xh.T)                     # [H, N]
    xlT = np.ascontiguousarray(xl.T)
    whT = np.ascontiguousarray(wh.T)                     # [H, E]
    wlT = np.ascontiguousarray(wl.T)

    iota = np.tile(np.arange(E, dtype=np.float32), (P, J))
    ones = np.ones((P, 1), np.float32)
    eye = np.eye(E, dtype=np.float32)
    ident = np.ascontiguousarray(np.vstack([eye, eye]))  # both partition halves

    ntok = x.shape[0]
    per_core = ntok // NCORES
    in_maps = []
    for c in range(NCORES):
        sl = slice(c * per_core, (c + 1) * per_core)
        in_maps.append({
            "xh": np.ascontiguousarray(xhT[:, sl]),
            "xl": np.ascontiguousarray(xlT[:, sl]),
            "wh": whT,
            "wl": wlT,
            "iota": iota,
            "ones": ones,
            "ident": ident,
        })
    return in_maps


def assemble_outputs(tokouts, stats, zs):
    """Host epilogue: concat per-token outputs, reduce per-core stat partials."""
    tok = np.concatenate(tokouts, axis=0)                # [N, 4]
    routing_weights = np.ascontiguousarray(tok[:, 0:2], dtype=np.float32)
    selected_experts = np.rint(tok[:, 2:4]).astype(np.int32)

    st = np.sum(np.stack([s.reshape(-1) for s in stats], 0), axis=0, dtype=np.float64)
    counts = st[0:E]
    probsum = st[E:2 * E]
    logz = np.log(np.concatenate([z.reshape(-1) for z in zs]).astype(np.float64))
    lse2sum = np.sum(logz * logz)
    ntok = float(tok.shape[0])

    tokens_per_expert = counts / ntok
    prob_per_expert = probsum / ntok
    load_balancing_loss = np.float32(E * np.sum(tokens_per_expert * prob_per_expert))
    z_loss = np.float32(lse2sum / ntok)
    utilization = (counts / ntok).astype(np.float32)
    return routing_weights, selected_experts, load_balancing_loss, z_loss, utilization


_NC_CACHE = {}


def _get_module():
    if "nc" not in _NC_CACHE:
        _NC_CACHE["nc"] = build_module(NT, debug=False)
    return _NC_CACHE["nc"]


def run(inputs, trace=False, **kwargs):
    """Run on hardware; returns (outputs_tuple, BassKernelResults)."""
    in_maps = host_prepare(inputs["hidden_states"], inputs["W_gate"])
    nc = _get_module()
    res = run_bass_kernel_spmd(
        nc, in_maps, core_ids=list(range(NCORES)), trace=trace, **kwargs
    )
    tokouts = [res.results[c]["tokout"] for c in range(NCORES)]
    stats = [res.results[c]["stats"] for c in range(NCORES)]
    zs = [res.results[c]["zout"] for c in range(NCORES)]
    return assemble_outputs(tokouts, stats, zs), res


def kernel(**inputs):
    outs, _ = run(inputs, trace=False)
    return outs
